# revision 6
# baseline (speedup 1.0000x reference)
"""Trainium2 8-core kernel for nn_Attention_55070070670307.

Reference model: per-head Cayley-orthogonalized projections (OrthogonLin)
feeding standard multi-head softmax attention.

  x: (2, 4096, 512) f32, 8 heads x 64 dim, Wq/Wk/Wv/Wo (512,512) + scalars
  aq/ak/av/ao + bias bo.

Strategy:
  * Host: Cayley-orthogonalize the four weight matrices per head (32 tiny
    64x64 solves -- negligible FLOPs, done in float64 numpy).
  * Device sharding: batch-parallel x head-parallel. Core c handles batch
    b = c//4 and heads {2*(c%4), 2*(c%4)+1}. Each core computes q/k/v
    projections for its 2 heads over the whole sequence (4096), full
    softmax attention per head, and the partial output projection
    (contribution of its 128 head-dims to all 512 output features).
  * The 4 cores of each batch group ReduceScatter the partial outputs
    (per 512-row chunk, overlapped with remaining compute), add bias,
    and write disjoint row-slices of the final output.

Device layouts (per core):
  xT   (512, 4096)  x[b] transposed (feature-major)       -> bf16 on chip
  qT/kT (128, 4096)  per-head-dim-major projections, bf16
  v    32 tiles (128n, 130) = [v_h0 | ones | v_h1 | ones] bf16 (ones col
       gives the softmax row-sum for free during the AV matmul)
  scores are computed transposed: sT (128k, 512q) = K_tile @ qT so that
  exp(sT) tiles feed the AV matmul as lhsT with zero transposes.
  Softmax uses the unnormalized trick: out = (exp(s) @ [v|1]); divide by
  the ones-column afterwards. No max-subtraction (scores*0.125 is in
  [-6, 6] comfortably for exp in f32).
"""

import os
import sys

import numpy as np

sys.path.insert(0, "/opt/trn_rl_repo")

HEADS = 8
DIM = 512
DH = 64  # dim per head
N = 4096  # sequence length
B = 2
SCALE = DH ** -0.5
NCORES = 8

F32 = None  # set lazily after mybir import
BF16 = None

_CACHE = {}
LAST_RESULT = None  # BassKernelResults of the most recent run (for test.py)


# ----------------------------------------------------------------------------
# Host-side Cayley orthogonalization (matches reference.cayley_heads, f64)
# ----------------------------------------------------------------------------
def cayley_heads_np(W: np.ndarray, alpha: float) -> np.ndarray:
    W = W.astype(np.float64)
    out, inn = W.shape
    d = inn // HEADS
    Wh = W.reshape(HEADS, d, inn)
    norms = np.sqrt((Wh * Wh).sum(axis=(1, 2), keepdims=True))
    Wn = float(alpha) * Wh / norms
    blocks = []
    I = np.eye(d)
    for j in range(HEADS):
        Wt = Wn[j].T  # (inn, d)
        U, V = Wt[:d], Wt[d:]
        A = U - U.T + V.T @ V
        IpA = I + A
        top = np.linalg.solve(IpA, I - A)
        bot = -2.0 * np.linalg.solve(IpA.T, V.T).T
        blocks.append(np.concatenate([top, bot], axis=0).T)  # (d, inn)
    return np.concatenate(blocks, axis=0)  # (out, inn) f64


# ----------------------------------------------------------------------------
# Device kernel builder (one SPMD graph, 8 cores)
# ----------------------------------------------------------------------------
def _build():
    from concourse import bass, bacc, tile
    import concourse.mybir as mybir

    F32 = mybir.dt.float32
    BF16 = mybir.dt.bfloat16
    EXP = mybir.ActivationFunctionType.Exp

    nc = bacc.Bacc(None, target_bir_lowering=False, debug=False, num_devices=NCORES)

    xT_e = nc.declare_dram_parameter("xT", [DIM, N], F32, isOutput=False)
    wq_e = nc.declare_dram_parameter("wq", [DIM, 128], F32, isOutput=False)
    wk_e = nc.declare_dram_parameter("wk", [DIM, 128], F32, isOutput=False)
    wv_e = nc.declare_dram_parameter("wv", [DIM, 128], F32, isOutput=False)
    wo_e = nc.declare_dram_parameter("wo", [128, DIM], F32, isOutput=False)
    bo_e = nc.declare_dram_parameter("bo", [1, DIM], F32, isOutput=False)
    out_e = nc.declare_dram_parameter("out", [8, 128, DIM], F32, isOutput=True)

    NKT = N // 128        # 32 k tiles
    NQB = N // 512        # 8 q blocks (512 wide)
    VW = 130              # v tile width: 64 + 1 + 64 + 1

    with tile.TileContext(nc) as tc:
        with (
            tc.tile_pool(name="persist", bufs=1) as persist,
            tc.tile_pool(name="stage", bufs=2) as stage,
            tc.tile_pool(name="es", bufs=3) as esp,
            tc.tile_pool(name="small", bufs=3) as small,
            tc.tile_pool(name="fo", bufs=3) as fop,
            tc.tile_pool(name="ps_big", bufs=2, space="PSUM") as ps_big,
            tc.tile_pool(name="ps_o", bufs=2, space="PSUM") as ps_o,
            tc.tile_pool(name="ps_f", bufs=2, space="PSUM") as ps_f,
            tc.tile_pool(name="dram", bufs=3, space="DRAM") as dram,
        ):
            # ---------------- weights + bias ----------------
            w32 = stage.tile([128, 512], F32, tag="w32")
            wqb = persist.tile([128, 512], BF16, tag="wqb")
            nc.sync.dma_start(w32[:].rearrange("p (c h) -> p c h", h=128), wq_e[:].rearrange("(c p) h -> p c h", p=128))
            nc.vector.tensor_copy(wqb[:], w32[:])
            w32 = stage.tile([128, 512], F32, tag="w32")
            wkb = persist.tile([128, 512], BF16, tag="wkb")
            nc.sync.dma_start(w32[:].rearrange("p (c h) -> p c h", h=128), wk_e[:].rearrange("(c p) h -> p c h", p=128))
            nc.vector.tensor_copy(wkb[:], w32[:])
            w32 = stage.tile([128, 512], F32, tag="w32")
            wvb = persist.tile([128, 512], BF16, tag="wvb")
            nc.sync.dma_start(w32[:].rearrange("p (c h) -> p c h", h=128), wv_e[:].rearrange("(c p) h -> p c h", p=128))
            nc.vector.tensor_copy(wvb[:], w32[:])
            w32 = stage.tile([128, 512], F32, tag="w32")
            wob = persist.tile([128, 512], BF16, tag="wob")
            nc.sync.dma_start(w32[:], wo_e[:])
            nc.vector.tensor_copy(wob[:], w32[:])

            bo1 = persist.tile([1, 512], F32, tag="bo1")
            nc.sync.dma_start(bo1[:], bo_e[:])
            bob = persist.tile([128, 512], F32, tag="bob")
            nc.gpsimd.partition_broadcast(bob[:], bo1[:])

            # ---------------- load x, cast to bf16 ----------------
            xbf = persist.tile([128, 4 * N], BF16, tag="xbf")  # 4 chunks of 4096
            for c in range(4):
                x32 = stage.tile([128, N], F32, tag="x32")
                nc.sync.dma_start(x32[:], xT_e[c * 128:(c + 1) * 128, :])
                nc.vector.tensor_copy(xbf[:, c * N:(c + 1) * N], x32[:])

            # ---------------- projections ----------------
            qT = persist.tile([128, N], BF16, tag="qT")
            kT = persist.tile([128, N], BF16, tag="kT")
            for dst, w in ((qT, wqb), (kT, wkb)):
                for f in range(4):  # 1024-wide free chunks
                    ps = ps_big.tile([128, 1024], F32, tag="ps_big")
                    for c in range(4):
                        nc.tensor.matmul(
                            ps[:, 0:512],
                            w[:, c * 128:(c + 1) * 128],
                            xbf[:, c * N + f * 1024: c * N + f * 1024 + 512],
                            start=(c == 0), stop=(c == 3),
                        )
                    for c in range(4):
                        nc.tensor.matmul(
                            ps[:, 512:1024],
                            w[:, c * 128:(c + 1) * 128],
                            xbf[:, c * N + f * 1024 + 512: c * N + (f + 1) * 1024],
                            start=(c == 0), stop=(c == 3),
                        )
                    nc.vector.tensor_copy(dst[:, f * 1024:(f + 1) * 1024], ps[:])

            vsb = persist.tile([128, NKT * VW], BF16, tag="vsb")
            nc.vector.memset(vsb[:], 1.0)
            for t0 in range(0, NKT, 4):  # 4 v tiles per PSUM (128,128) quarter
                ps = ps_big.tile([128, 1024], F32, tag="ps_big")
                for i in range(4):
                    t = t0 + i
                    for c in range(4):
                        nc.tensor.matmul(
                            ps[:, i * 128:(i + 1) * 128],
                            xbf[:, c * N + t * 128: c * N + (t + 1) * 128],
                            wvb[:, c * 128:(c + 1) * 128],
                            start=(c == 0), stop=(c == 3),
                        )
                for i in range(4):
                    t = t0 + i
                    nc.vector.tensor_copy(
                        vsb[:, t * VW: t * VW + 64], ps[:, i * 128: i * 128 + 64])
                    nc.vector.tensor_copy(
                        vsb[:, t * VW + 65: t * VW + 129], ps[:, i * 128 + 64: (i + 1) * 128])

            # ---------------- attention + output projection ----------------
            outT = persist.tile([128, N], BF16, tag="outT")  # normalized attn out.T
            for qb in range(NQB):
                q0 = qb * 512
                for h in range(2):
                    po = ps_o.tile([65, 512], F32, tag="ps_o")
                    for kt2 in range(NKT // 2):  # pairs of k tiles
                        ps = ps_big.tile([128, 1024], F32, tag="ps_big")
                        for j in range(2):
                            kt = kt2 * 2 + j
                            nc.tensor.matmul(
                                ps[:, j * 512:(j + 1) * 512],
                                kT[h * 64:(h + 1) * 64, kt * 128:(kt + 1) * 128],
                                qT[h * 64:(h + 1) * 64, q0:q0 + 512],
                                start=True, stop=True,
                            )
                        es = esp.tile([128, 1024], BF16, tag="es")
                        nc.scalar.activation(es[:], ps[:], EXP, scale=SCALE)
                        for j in range(2):
                            kt = kt2 * 2 + j
                            nc.tensor.matmul(
                                po[:],
                                vsb[:, kt * VW + 65 * h: kt * VW + 65 * h + 65],
                                es[:, j * 512:(j + 1) * 512],
                                start=(kt == 0), stop=(kt == NKT - 1),
                            )
                    # normalize: outT[h] = po[:64] * (1 / po[64])
                    rc = small.tile([1, 512], F32, tag="rc")
                    nc.vector.reciprocal(rc[:], po[64:65, :])
                    rb = small.tile([64, 512], F32, tag="rb")
                    nc.gpsimd.partition_broadcast(rb[:], rc[:])
                    nc.vector.tensor_mul(
                        outT[h * 64:(h + 1) * 64, q0:q0 + 512], po[0:64, :], rb[:])

                # partial output projection for this 512-q block
                part = dram.tile([512, DIM], F32, tag="part")
                for sub in range(4):
                    pf = ps_f.tile([128, 512], F32, tag="ps_f")
                    nc.tensor.matmul(
                        pf[:], outT[:, q0 + sub * 128: q0 + (sub + 1) * 128],
                        wob[:], start=True, stop=True)
                    fo = fop.tile([128, 512], F32, tag="fo")
                    nc.vector.tensor_copy(fo[:], pf[:])
                    nc.sync.dma_start(part[sub * 128:(sub + 1) * 128, :], fo[:])

                rs = dram.tile([128, DIM], F32, tag="rs")
                nc.gpsimd.collective_compute(
                    "ReduceScatter",
                    mybir.AluOpType.add,
                    replica_groups=[[0, 1, 2, 3], [4, 5, 6, 7]],
                    ins=[part.opt()],
                    outs=[rs.opt()],
                )
                rsb = fop.tile([128, 512], F32, tag="rsb")
                nc.sync.dma_start(rsb[:], rs[:])
                ob = fop.tile([128, 512], F32, tag="ob")
                nc.vector.tensor_add(ob[:], rsb[:], bob[:])
                nc.sync.dma_start(out_e[qb], ob[:])

    nc.compile()
    return nc


def _get_nc():
    if "nc" not in _CACHE:
        _CACHE["nc"] = _build()
    return _CACHE["nc"]


# ----------------------------------------------------------------------------
# PJRT runner (mirrors bass2jax.run_bass_via_pjrt multi-core branch, but keeps
# the jitted callable cached so repeated calls / benchmarking don't recompile)
# ----------------------------------------------------------------------------
def _pjrt_exec(nc, in_maps, bench_iters=0):
    import jax
    import numpy as _np
    from jax.sharding import Mesh, PartitionSpec, NamedSharding
    from jax.experimental.shard_map import shard_map
    import concourse.mybir as mybir
    from concourse import bass2jax

    bass2jax.install_neuronx_cc_hook()

    n_cores = NCORES
    if "runner" not in _CACHE:
        pname = nc.partition_id_tensor.name if nc.partition_id_tensor else None
        in_names, out_names, out_avals, zero_outs = [], [], [], []
        for alloc in nc.m.functions[0].allocations:
            if not isinstance(alloc, mybir.MemoryLocationSet):
                continue
            name = alloc.memorylocations[0].name
            if alloc.kind == "ExternalInput":
                if name != pname:
                    in_names.append(name)
            elif alloc.kind == "ExternalOutput":
                sh = tuple(alloc.tensor_shape)
                dt = mybir.dt.np(alloc.dtype)
                out_names.append(name)
                out_avals.append(jax.core.ShapedArray(sh, dt))
                zero_outs.append(_np.zeros(sh, dt))
        n_params = len(in_names)
        n_outs = len(out_avals)
        all_names = in_names + out_names + ([pname] if pname else [])

        def _body(*args):
            operands = list(args)
            if pname is not None:
                operands.append(bass2jax.partition_id_tensor())
            outs = bass2jax._bass_exec_p.bind(
                *operands,
                out_avals=tuple(out_avals),
                in_names=tuple(all_names),
                out_names=tuple(out_names),
                lowering_input_output_aliases=(),
                sim_require_finite=True,
                sim_require_nnan=True,
                nc=nc,
            )
            return tuple(outs)

        donate = tuple(range(n_params, n_params + n_outs))
        devices = jax.devices()[:n_cores]
        mesh = Mesh(_np.asarray(devices), ("core",))
        in_specs = (PartitionSpec("core"),) * (n_params + n_outs)
        out_specs = (PartitionSpec("core"),) * n_outs
        sharded = jax.jit(
            shard_map(_body, mesh=mesh, in_specs=in_specs, out_specs=out_specs,
                      check_rep=False),
            donate_argnums=donate, keep_unused=True)
        _CACHE["runner"] = (sharded, in_names, out_names, out_avals, zero_outs, mesh)

    sharded, in_names, out_names, out_avals, zero_outs, mesh = _CACHE["runner"]
    shd = NamedSharding(mesh, PartitionSpec("core"))

    concat_in = [
        jax.device_put(
            _np.concatenate([_np.asarray(m[nm]) for m in in_maps], axis=0), shd)
        for nm in in_names
    ]
    def zeros_dev():
        return [jax.device_put(
            _np.zeros((n_cores * z.shape[0], *z.shape[1:]), z.dtype), shd)
            for z in zero_outs]

    out_arrs = sharded(*concat_in, *zeros_dev())
    jax.block_until_ready(out_arrs)

    per_iter_ns = None
    if bench_iters > 0:
        import time as _time
        zs = [zeros_dev() for _ in range(bench_iters)]
        # warmup a couple extra dispatches
        for z in zs[:2]:
            o = sharded(*concat_in, *z)
        jax.block_until_ready(o)
        zs = [zeros_dev() for _ in range(bench_iters)]
        t0 = _time.perf_counter()
        for z in zs:
            o = sharded(*concat_in, *z)
        jax.block_until_ready(o)
        t1 = _time.perf_counter()
        per_iter_ns = (t1 - t0) / bench_iters * 1e9

    results = [
        {nm: _np.asarray(out_arrs[i]).reshape(n_cores, *out_avals[i].shape)[c]
         for i, nm in enumerate(out_names)}
        for c in range(n_cores)
    ]
    return results, per_iter_ns


# ----------------------------------------------------------------------------
# Entry point
# ----------------------------------------------------------------------------
def kernel(x, Wq, aq, Wk, ak, Wv, av, Wo, ao, bo):
    global LAST_RESULT

    x = np.asarray(x, dtype=np.float32)
    Qq = cayley_heads_np(np.asarray(Wq), float(aq))
    Qk = cayley_heads_np(np.asarray(Wk), float(ak))
    Qv = cayley_heads_np(np.asarray(Wv), float(av))
    Qo = cayley_heads_np(np.asarray(Wo), float(ao))
    bo = np.asarray(bo, dtype=np.float32)

    nc = _get_nc()

    in_maps = []
    for c in range(NCORES):
        b = c // 4
        hp = c % 4
        sl = slice(hp * 128, (hp + 1) * 128)  # this core's two heads' dims
        in_maps.append({
            "xT": np.ascontiguousarray(x[b].T),                       # (512, 4096)
            "wq": np.ascontiguousarray(Qq[sl].T).astype(np.float32),  # (512, 128)
            "wk": np.ascontiguousarray(Qk[sl].T).astype(np.float32),
            "wv": np.ascontiguousarray(Qv[sl].T).astype(np.float32),
            "wo": np.ascontiguousarray(Qo[:, sl].T).astype(np.float32),  # (128, 512)
            "bo": bo.reshape(1, DIM),
        })

    bench_iters = int(os.environ.get("KERNEL_BENCH", "0"))
    results, per_iter_ns = _pjrt_exec(nc, in_maps, bench_iters=bench_iters)
    LAST_RESULT = {"per_iter_ns": per_iter_ns}

    out = np.empty((B, N, DIM), dtype=np.float32)
    for c in range(NCORES):
        b = c // 4
        r = c % 4
        oc = results[c]["out"]  # (8, 128, 512)
        for qb in range(8):
            out[b, qb * 512 + r * 128: qb * 512 + (r + 1) * 128, :] = oc[qb]
    return out
